# revision 15
# baseline (speedup 1.0000x reference)
"""Causal self-attention TRN2 Bass kernel (B=4, T=2048, C=1024, H=16, D=64, fp32).

Sharding: 8 cores = 4 batches x 2 head-groups (8 heads each). Each core computes
its batch's QKV for its heads, causal flash-style attention, and a partial
output projection; the host sums the two head-group partials per batch.

v2: fully SBUF-resident, fused qc-outer pipeline.
  Per 512-query chunk tch (=qc):
    QKV: q_t[pr], k_sb[:, pr, tch], va[kb] computed from streamed x chunk
         (PE matmuls f32r; Pool drains PSUM->SBUF with bias add)
    attention qc=tch for all 4 head-pairs pr (needs only keys <= chunk end):
         S^T[k,q] strips (f32r, diagonal strips padded to >=256 rows),
         causal mask via DVE add of NEG triangle, exp on ACT (scale=1/8,
         bf16 out), AV with [V|ones] stationary (bf16) accumulating O^T and
         softmax sums in one PSUM tile; Pool drains, DVE reciprocal,
         DVE/Pool multiply -> oT (bf16)
    proj(tch): y^T partial = W_proj^T oT (bf16 x bf16), Pool bias-drain,
         DMA out.
Host: y[b] = (yT[2b] + yT[2b+1]).T
"""

import numpy as np
from contextlib import ExitStack

import concourse.bass as bass
import concourse.tile as tile
from concourse import bacc, mybir
from concourse.bass import ts
from concourse.bass_utils import run_bass_kernel_spmd

N_CORES = 8
B, T, C, H, D = 4, 2048, 1024, 16, 64
CB = C // 128          # 8 contraction blocks
NEG = -1.0e9

F32 = mybir.dt.float32
F32R = mybir.dt.float32r
BF16 = mybir.dt.bfloat16
AF = mybir.ActivationFunctionType
OP = mybir.AluOpType

_CACHE = {}

# query-strip low offset by diagonal position r (r = kb - 4*qc; r<0 off-diag)
_QLO = {0: 0, 1: 128, 2: 256, 3: 256}


def _build(phases=(1, 2, 3), reps=1):
    nc = bacc.Bacc("TRN2", target_bir_lowering=False, debug=False, num_devices=N_CORES)

    xT = nc.dram_tensor("xT", [C, T], F32R, kind="ExternalInput").ap()
    w_qk = nc.dram_tensor("w_qk", [C, 1024], F32R, kind="ExternalInput").ap()
    w_v = nc.dram_tensor("w_v", [C, 512], F32R, kind="ExternalInput").ap()
    w_pr = nc.dram_tensor("w_pr", [512, C], BF16, kind="ExternalInput").ap()
    b_qk = nc.dram_tensor("b_qk", [1024], F32, kind="ExternalInput").ap()
    b_v = nc.dram_tensor("b_v", [128, 512], F32, kind="ExternalInput").ap()
    b_pr = nc.dram_tensor("b_pr", [C], F32, kind="ExternalInput").ap()
    yT = nc.dram_tensor("yT", [C, T], F32, kind="ExternalOutput").ap()

    xT_r = xT.rearrange("(cb p) t -> p cb t", p=128)
    w_qk_r = w_qk.rearrange("(cb p) m -> p cb m", p=128)
    w_v_r = w_v.rearrange("(cb p) m -> p cb m", p=128)
    w_pr_r = w_pr.rearrange("(pb p) m -> p pb m", p=128)
    b_qk_r = b_qk.rearrange("(m p) -> p m", p=128)
    b_pr_r = b_pr.rearrange("(m p) -> p m", p=128)
    yT_r = yT.rearrange("(m p) t -> p m t", p=128)

    with tile.TileContext(nc) as tc:
        with ExitStack() as ctx:
            wqk_p = ctx.enter_context(tc.tile_pool(name="wqk", bufs=1))
            w2_p = ctx.enter_context(tc.tile_pool(name="w2", bufs=1))
            wpr_p = ctx.enter_context(tc.tile_pool(name="wpr", bufs=1))
            k_pl = ctx.enter_context(tc.tile_pool(name="kp", bufs=1))
            va_pl = ctx.enter_context(tc.tile_pool(name="vap", bufs=1))
            ot_pl = ctx.enter_context(tc.tile_pool(name="otp", bufs=1))
            x_pl = ctx.enter_context(tc.tile_pool(name="xp", bufs=2))
            q_pl = ctx.enter_context(tc.tile_pool(name="qp", bufs=2))
            p_pl = ctx.enter_context(tc.tile_pool(name="pp", bufs=4))
            ys_pl = ctx.enter_context(tc.tile_pool(name="ysp", bufs=2))
            rc_pl = ctx.enter_context(tc.tile_pool(name="rcp", bufs=1))
            misc = ctx.enter_context(tc.tile_pool(name="misc", bufs=1))
            ps_s = ctx.enter_context(tc.tile_pool(name="ps_s", bufs=3, space="PSUM"))
            ps_o = ctx.enter_context(tc.tile_pool(name="ps_o", bufs=1, space="PSUM"))

            # constants
            b_qk_sb = misc.tile([128, 8], F32)
            nc.sync.dma_start(b_qk_sb[:], b_qk_r)
            b_v_sb = misc.tile([128, 512], F32)
            nc.sync.dma_start(b_v_sb[:], b_v)
            b_pr_sb = misc.tile([128, 8], F32)
            nc.sync.dma_start(b_pr_sb[:], b_pr_r)
            # tri2: [128k, 256q] bf16 0/1: cols 0:128 all 0, cols 128:256
            # 1 where q>=k else 0 (q_rel = col-128, k = partition)
            tri2 = misc.tile([128, 256], BF16)
            nc.gpsimd.memset(tri2[:], 1.0)
            nc.gpsimd.affine_select(
                out=tri2[:], in_=tri2[:], compare_op=OP.is_ge, fill=0.0,
                base=-128, pattern=[[1, 256]], channel_multiplier=-1,
            )
            tri = tri2[:, 128:256]

            # weights
            w_qk_sb = wqk_p.tile([128, CB, 1024], F32R)
            nc.sync.dma_start(w_qk_sb[:], w_qk_r)
            w_v_sb = w2_p.tile([128, CB, 512], F32R)
            nc.sync.dma_start(w_v_sb[:], w_v_r)
            w_pr_sb = wpr_p.tile([128, 4, 1024], BF16)
            nc.sync.dma_start(w_pr_sb[:], w_pr_r)

            # persistent activations (shared across reps; rewritten per rep)
            k_sb = k_pl.tile([128, 4, T], F32R, name="k_sb")
            va = va_pl.tile([128, 16, 8, 2, 64], BF16, name="va")
            oT = ot_pl.tile([128, 4, T], BF16, name="oT")
            nc.gpsimd.memset(va[:, :, :, 1, :], 1.0)

            for _rep in range(reps):
                # Cycle-shifted pipeline: cycle c emits attention(qc=c-1)
                # with QKV(c) and proj(c-2) work-units interleaved into the
                # PE stream as gap fill; x(c+1) prefetched at cycle start.
                x_ts = {}
                q_ts = {}
                fill = []

                def qkv_units(tch):
                    x_t = x_ts[tch]
                    q_t = q_ts[tch]
                    units = []
                    for mp in range(4):
                        def u(mp=mp, tch=tch, x_t=x_t, q_t=q_t):
                            ps = ps_s.tile([128, 2, 512], F32, tag="ps_s",
                                           name=f"qk_{_rep}_{tch}_{mp}")
                            for cb in range(CB):
                                for h in (0, 1):
                                    nc.tensor.matmul(
                                        ps[:, h],
                                        w_qk_sb[:, cb, ts(2 * mp + h, 128)],
                                        x_t[:, cb],
                                        start=(cb == 0), stop=(cb == CB - 1),
                                    )
                            for h in (0, 1):
                                m = 2 * mp + h
                                if m < 4:
                                    nc.vector.tensor_scalar_add(
                                        q_t[:, m, :], ps[:, h],
                                        b_qk_sb[:, m : m + 1])
                                else:
                                    nc.vector.tensor_scalar_add(
                                        k_sb[:, m - 4, ts(tch, 512)], ps[:, h],
                                        b_qk_sb[:, m : m + 1])
                        units.append(u)
                    for vp in range(2):
                        def u(vp=vp, tch=tch, x_t=x_t):
                            ps = ps_s.tile([128, 2, 512], F32, tag="ps_s",
                                           name=f"v_{_rep}_{tch}_{vp}")
                            for cb in range(CB):
                                for h in (0, 1):
                                    nc.tensor.matmul(
                                        ps[:, h],
                                        x_t[:, cb, ts(2 * vp + h, 128)],
                                        w_v_sb[:, cb],
                                        start=(cb == 0), stop=(cb == CB - 1),
                                    )
                            for h in (0, 1):
                                tb = tch * 4 + 2 * vp + h
                                nc.vector.tensor_tensor(
                                    va[:, tb, :, 0, :],
                                    ps[:, h].rearrange("p (s d) -> p s d", s=8),
                                    b_v_sb.rearrange("p (s d) -> p s d", s=8),
                                    OP.add)
                        units.append(u)
                    return units

                def proj_units(qc):
                    units = []
                    for mp in range(4):
                        def u(mp=mp, qc=qc):
                            ps = ps_s.tile([128, 2, 512], F32, tag="ps_s",
                                           name=f"y_{_rep}_{qc}_{mp}")
                            for h in (0, 1):
                                for pb in range(4):
                                    nc.tensor.matmul(
                                        ps[:, h],
                                        w_pr_sb[:, pb, ts(2 * mp + h, 128)],
                                        oT[:, pb, ts(qc, 512)],
                                        start=(pb == 0), stop=(pb == 3),
                                    )
                            yst = ys_pl.tile([128, 2, 512], F32, tag="ystg",
                                             name=f"yst_{_rep}_{qc}_{mp}")
                            for h in (0, 1):
                                nc.vector.tensor_scalar_add(
                                    yst[:, h], ps[:, h],
                                    b_pr_sb[:, 2 * mp + h : 2 * mp + h + 1])
                            nc.sync.dma_start(
                                yT_r[:, 2 * mp : 2 * mp + 2, ts(qc, 512)],
                                yst[:])
                        units.append(u)
                    return units

                def attention(qc, pump):
                    q_t = q_ts[qc]
                    nkb = 4 * qc + 4
                    for pr in range(4):
                        po = ps_o.tile([128, 2, 512], F32, tag="ps_o",
                                       name=f"o_{_rep}_{qc}_{pr}")
                        s_tiles = {}
                        p_tiles = {}

                        def s_step(kb, pr=pr, q_t=q_t, s_tiles=s_tiles):
                            r = kb - 4 * qc
                            qlo = _QLO.get(r, 0)
                            s = ps_s.tile([128, 2, 512], F32, tag="ps_s",
                                          name=f"s_{_rep}_{qc}_{pr}_{kb}")
                            for j in (0, 1):
                                pb = 64 * j
                                nc.tensor.matmul(
                                    s[:, j, qlo:512],
                                    k_sb[pb : pb + 64, pr, ts(kb, 128)],
                                    q_t[pb : pb + 64, pr, qlo:512],
                                    start=True, stop=True,
                                    tile_position=(pb, 0),
                                )
                            s_tiles[kb] = (s, qlo)

                        def exp_step(kb, s_tiles=s_tiles, p_tiles=p_tiles,
                                     pr=pr):
                            s, qlo = s_tiles.pop(kb)
                            r = kb - 4 * qc
                            p_t = p_pl.tile([128, 2, 512], BF16, tag="p",
                                            name=f"p_{_rep}_{qc}_{pr}_{kb}")
                            nc.scalar.activation(
                                p_t[:, :, qlo:512], s[:, :, qlo:512],
                                AF.Exp, scale=0.125)
                            if 0 <= r <= 2:
                                c0 = 128 * r
                                nc.gpsimd.tensor_tensor(
                                    p_t[:, :, c0 : c0 + 128],
                                    p_t[:, :, c0 : c0 + 128],
                                    tri[:, None, :].to_broadcast((128, 2, 128)),
                                    OP.mult)
                            elif r == 3:
                                nc.gpsimd.tensor_tensor(
                                    p_t[:, :, 256:512], p_t[:, :, 256:512],
                                    tri2[:, None, :].to_broadcast((128, 2, 256)),
                                    OP.mult)
                            p_tiles[kb] = (p_t, qlo)

                        def av_step(kb, pr=pr, po=po, p_tiles=p_tiles,
                                    nkb=nkb):
                            p_t, qlo = p_tiles.pop(kb)
                            for j in (0, 1):
                                s0 = 2 * pr + j
                                nc.tensor.matmul(
                                    po[:, j, qlo:512],
                                    va[:, kb, s0].rearrange("p a b -> p (a b)"),
                                    p_t[:, j, qlo:512],
                                    start=(kb == 0), stop=(kb == nkb - 1),
                                )

                        s_step(0)
                        if nkb > 1:
                            s_step(1)
                        exp_step(0)
                        if nkb > 2:
                            s_step(2)
                        if nkb > 1:
                            exp_step(1)
                        for kb in range(nkb):
                            if kb + 3 < nkb:
                                s_step(kb + 3)
                            if kb + 2 < nkb:
                                exp_step(kb + 2)
                            av_step(kb)
                            if kb + 1 < nkb:
                                pump()

                        rec = rc_pl.tile([64, 2, 512], F32, tag="rec",
                                         name=f"rec_{_rep}_{qc}_{pr}")
                        nc.vector.reciprocal(rec[:], po[64:128])
                        for j in (0, 1):
                            nc.vector.tensor_tensor(
                                oT[j * 64 : (j + 1) * 64, pr, ts(qc, 512)],
                                po[0:64, j], rec[:, j], OP.mult)
                        pump()

                # prologue: load x(0)
                x_ts[0] = x_pl.tile([128, CB, 512], F32R, tag="x",
                                    name=f"x_{_rep}_0")
                nc.sync.dma_start(x_ts[0][:], xT_r[:, :, ts(0, 512)])

                for c in range(6):
                    if c + 1 <= 3:
                        x_ts[c + 1] = x_pl.tile([128, CB, 512], F32R, tag="x",
                                                name=f"x_{_rep}_{c + 1}")
                        nc.sync.dma_start(x_ts[c + 1][:],
                                          xT_r[:, :, ts(c + 1, 512)])
                    if c <= 3:
                        q_ts[c] = q_pl.tile([128, 4, 512], F32R, tag="q",
                                            name=f"q_{_rep}_{c}")
                        fill.extend(qkv_units(c))
                    if 2 <= c <= 5:
                        fill.extend(proj_units(c - 2))
                    if 1 <= c <= 4:
                        qc = c - 1
                        steps = 4 * (4 * qc + 4) + 4
                        per = len(fill) / steps
                        state = {"acc": 0.0, "done": 0}

                        def pump(state=state, per=per):
                            state["acc"] += per
                            while state["done"] < int(state["acc"]) and fill:
                                fill.pop(0)()
                                state["done"] += 1

                        attention(qc, pump)
                    while fill:
                        fill.pop(0)()

    nc.compile()
    return nc


def _in_maps(x, W_attn, b_attn, W_proj, b_proj):
    maps = []
    for b in range(B):
        for g in range(2):
            cs = slice(g * 512, (g + 1) * 512)
            maps.append({
                "xT": np.ascontiguousarray(x[b].T),
                "w_qk": np.ascontiguousarray(
                    np.concatenate([W_attn[:, cs], W_attn[:, 1024 + cs.start : 1024 + cs.stop]], axis=1)),
                "w_v": np.ascontiguousarray(W_attn[:, 2048 + cs.start : 2048 + cs.stop]),
                "w_pr": np.ascontiguousarray(W_proj[cs, :]).astype(mybir.dt.np(BF16)),
                "b_qk": np.ascontiguousarray(
                    np.concatenate([b_attn[cs], b_attn[1024 + cs.start : 1024 + cs.stop]])),
                "b_v": np.ascontiguousarray(
                    np.tile(b_attn[2048 + cs.start : 2048 + cs.stop][None, :], (128, 1))),
                "b_pr": np.ascontiguousarray(b_proj),
            })
    return maps


def kernel(x, W_attn, b_attn, W_proj, b_proj):
    x = np.asarray(x, dtype=np.float32)
    W_attn = np.asarray(W_attn, dtype=np.float32)
    b_attn = np.asarray(b_attn, dtype=np.float32)
    W_proj = np.asarray(W_proj, dtype=np.float32)
    b_proj = np.asarray(b_proj, dtype=np.float32)

    if "nc" not in _CACHE:
        _CACHE["nc"] = _build()
    nc = _CACHE["nc"]

    maps = _in_maps(x, W_attn, b_attn, W_proj, b_proj)
    last_exc = None
    for attempt in range(3):
        try:
            res = run_bass_kernel_spmd(nc, maps, core_ids=list(range(N_CORES)))
            break
        except Exception as exc:  # transient device wedges recover on retry
            last_exc = exc
            if attempt == 2:
                raise
            import time as _time
            _time.sleep(5)
    y = np.empty((B, T, C), dtype=np.float32)
    for b in range(B):
        y[b] = (res.results[2 * b]["yT"] + res.results[2 * b + 1]["yT"]).T
    return y


# revision 18
# speedup vs baseline: 1.3305x; 1.3305x over previous
"""Causal self-attention TRN2 Bass kernel (B=4, T=2048, C=1024, H=16, D=64, fp32).

Sharding: 8 cores = 4 batches x 2 head-groups (8 heads each). Each core computes
its batch's QKV for its heads, causal flash-style attention, and a partial
output projection; the host sums the two head-group partials per batch.

v2: fully SBUF-resident, fused qc-outer pipeline.
  Per 512-query chunk tch (=qc):
    QKV: q_t[pr], k_sb[:, pr, tch], va[kb] computed from streamed x chunk
         (PE matmuls f32r; Pool drains PSUM->SBUF with bias add)
    attention qc=tch for all 4 head-pairs pr (needs only keys <= chunk end):
         S^T[k,q] strips (f32r, diagonal strips padded to >=256 rows),
         causal mask via DVE add of NEG triangle, exp on ACT (scale=1/8,
         bf16 out), AV with [V|ones] stationary (bf16) accumulating O^T and
         softmax sums in one PSUM tile; Pool drains, DVE reciprocal,
         DVE/Pool multiply -> oT (bf16)
    proj(tch): y^T partial = W_proj^T oT (bf16 x bf16), Pool bias-drain,
         DMA out.
Host: y[b] = (yT[2b] + yT[2b+1]).T
"""

import numpy as np
from contextlib import ExitStack

import concourse.bass as bass
import concourse.tile as tile
from concourse import bacc, mybir
from concourse.bass import ts
from concourse.bass_utils import run_bass_kernel_spmd

N_CORES = 8
B, T, C, H, D = 4, 2048, 1024, 16, 64
CB = C // 128          # 8 contraction blocks
NEG = -1.0e9

F32 = mybir.dt.float32
F32R = mybir.dt.float32r
BF16 = mybir.dt.bfloat16
AF = mybir.ActivationFunctionType
OP = mybir.AluOpType

_CACHE = {}

# query-strip low offset by diagonal position r (r = kb - 4*qc; r<0 off-diag)
_QLO = {0: 0, 1: 128, 2: 256, 3: 256}


def _build(phases=(1, 2, 3), reps=1):
    nc = bacc.Bacc("TRN2", target_bir_lowering=False, debug=False, num_devices=N_CORES)

    xT = nc.dram_tensor("xT", [C, T], F32R, kind="ExternalInput").ap()
    w_qk = nc.dram_tensor("w_qk", [C, 1024], F32R, kind="ExternalInput").ap()
    w_v = nc.dram_tensor("w_v", [C, 512], F32R, kind="ExternalInput").ap()
    w_pr = nc.dram_tensor("w_pr", [512, C], BF16, kind="ExternalInput").ap()
    b_qk = nc.dram_tensor("b_qk", [1024], F32, kind="ExternalInput").ap()
    b_v = nc.dram_tensor("b_v", [128, 512], F32, kind="ExternalInput").ap()
    b_pr = nc.dram_tensor("b_pr", [C], F32, kind="ExternalInput").ap()
    yT = nc.dram_tensor("yT", [C, T], F32, kind="ExternalOutput").ap()

    xT_r = xT.rearrange("(cb p) t -> p cb t", p=128)
    w_qk_r = w_qk.rearrange("(cb p) m -> p cb m", p=128)
    w_v_r = w_v.rearrange("(cb p) m -> p cb m", p=128)
    w_pr_r = w_pr.rearrange("(pb p) m -> p pb m", p=128)
    b_qk_r = b_qk.rearrange("(m p) -> p m", p=128)
    b_pr_r = b_pr.rearrange("(m p) -> p m", p=128)
    yT_r = yT.rearrange("(m p) t -> p m t", p=128)

    with tile.TileContext(nc) as tc:
        with ExitStack() as ctx:
            wqk_p = ctx.enter_context(tc.tile_pool(name="wqk", bufs=1))
            w2_p = ctx.enter_context(tc.tile_pool(name="w2", bufs=1))
            wpr_p = ctx.enter_context(tc.tile_pool(name="wpr", bufs=1))
            k_pl = ctx.enter_context(tc.tile_pool(name="kp", bufs=1))
            va_pl = ctx.enter_context(tc.tile_pool(name="vap", bufs=1))
            ot_pl = ctx.enter_context(tc.tile_pool(name="otp", bufs=1))
            x_pl = ctx.enter_context(tc.tile_pool(name="xp", bufs=2))
            q_pl = ctx.enter_context(tc.tile_pool(name="qp", bufs=2))
            p_pl = ctx.enter_context(tc.tile_pool(name="pp", bufs=4))
            ys_pl = ctx.enter_context(tc.tile_pool(name="ysp", bufs=2))
            rc_pl = ctx.enter_context(tc.tile_pool(name="rcp", bufs=1))
            misc = ctx.enter_context(tc.tile_pool(name="misc", bufs=1))
            ps_s = ctx.enter_context(tc.tile_pool(name="ps_s", bufs=3, space="PSUM"))
            ps_o = ctx.enter_context(tc.tile_pool(name="ps_o", bufs=1, space="PSUM"))

            # constants
            b_qk_sb = misc.tile([128, 8], F32)
            nc.sync.dma_start(b_qk_sb[:], b_qk_r)
            b_v_sb = misc.tile([128, 512], F32)
            nc.sync.dma_start(b_v_sb[:], b_v)
            b_pr_sb = misc.tile([128, 8], F32)
            nc.sync.dma_start(b_pr_sb[:], b_pr_r)
            # tri2: [128k, 256q] bf16 0/1: cols 0:128 all 0, cols 128:256
            # 1 where q>=k else 0 (q_rel = col-128, k = partition)
            tri2 = misc.tile([128, 256], BF16)
            nc.gpsimd.memset(tri2[:], 1.0)
            nc.gpsimd.affine_select(
                out=tri2[:], in_=tri2[:], compare_op=OP.is_ge, fill=0.0,
                base=-128, pattern=[[1, 256]], channel_multiplier=-1,
            )
            tri = tri2[:, 128:256]

            # weights
            w_qk_sb = wqk_p.tile([128, CB, 1024], F32R)
            nc.sync.dma_start(w_qk_sb[:], w_qk_r)
            w_v_sb = w2_p.tile([128, CB, 512], F32R)
            nc.sync.dma_start(w_v_sb[:], w_v_r)
            w_pr_sb = wpr_p.tile([128, 4, 1024], BF16)
            nc.sync.dma_start(w_pr_sb[:], w_pr_r)

            # persistent activations (shared across reps; rewritten per rep)
            k_sb = k_pl.tile([128, 4, T], F32R, name="k_sb")
            va = va_pl.tile([128, 16, 8, 2, 64], BF16, name="va")
            oT = ot_pl.tile([128, 4, T], BF16, name="oT")
            nc.gpsimd.memset(va[:, :, :, 1, :], 1.0)

            for _rep in range(reps):
                # Cycle-shifted pipeline: cycle c emits attention(qc=c-1)
                # with QKV(c) and proj(c-2) work-units interleaved into the
                # PE stream as gap fill; x(c+1) prefetched at cycle start.
                x_ts = {}
                q_ts = {}
                fill = []

                def qkv_units(tch):
                    x_t = x_ts[tch]
                    q_t = q_ts[tch]
                    units = []
                    for mp in range(4):
                        def u(mp=mp, tch=tch, x_t=x_t, q_t=q_t):
                            ps = ps_s.tile([128, 2, 512], F32, tag="ps_s",
                                           name=f"qk_{_rep}_{tch}_{mp}")
                            for cb in range(CB):
                                for h in (0, 1):
                                    nc.tensor.matmul(
                                        ps[:, h],
                                        w_qk_sb[:, cb, ts(2 * mp + h, 128)],
                                        x_t[:, cb],
                                        start=(cb == 0), stop=(cb == CB - 1),
                                    )
                            for h in (0, 1):
                                m = 2 * mp + h
                                if m < 4:
                                    nc.vector.tensor_scalar_add(
                                        q_t[:, m, :], ps[:, h],
                                        b_qk_sb[:, m : m + 1])
                                else:
                                    nc.vector.tensor_scalar_add(
                                        k_sb[:, m - 4, ts(tch, 512)], ps[:, h],
                                        b_qk_sb[:, m : m + 1])
                        units.append(u)
                    for vp in range(2):
                        def u(vp=vp, tch=tch, x_t=x_t):
                            ps = ps_s.tile([128, 2, 512], F32, tag="ps_s",
                                           name=f"v_{_rep}_{tch}_{vp}")
                            for cb in range(CB):
                                for h in (0, 1):
                                    nc.tensor.matmul(
                                        ps[:, h],
                                        x_t[:, cb, ts(2 * vp + h, 128)],
                                        w_v_sb[:, cb],
                                        start=(cb == 0), stop=(cb == CB - 1),
                                    )
                            for h in (0, 1):
                                tb = tch * 4 + 2 * vp + h
                                nc.vector.tensor_tensor(
                                    va[:, tb, :, 0, :],
                                    ps[:, h].rearrange("p (s d) -> p s d", s=8),
                                    b_v_sb.rearrange("p (s d) -> p s d", s=8),
                                    OP.add)
                        units.append(u)
                    return units

                def proj_units(qc):
                    units = []
                    for mp in range(4):
                        def u(mp=mp, qc=qc):
                            ps = ps_s.tile([128, 2, 512], F32, tag="ps_s",
                                           name=f"y_{_rep}_{qc}_{mp}")
                            for h in (0, 1):
                                for pb in range(4):
                                    nc.tensor.matmul(
                                        ps[:, h],
                                        w_pr_sb[:, pb, ts(2 * mp + h, 128)],
                                        oT[:, pb, ts(qc, 512)],
                                        start=(pb == 0), stop=(pb == 3),
                                    )
                            yst = ys_pl.tile([128, 2, 512], F32, tag="ystg",
                                             name=f"yst_{_rep}_{qc}_{mp}")
                            for h in (0, 1):
                                nc.vector.tensor_scalar_add(
                                    yst[:, h], ps[:, h],
                                    b_pr_sb[:, 2 * mp + h : 2 * mp + h + 1])
                            nc.sync.dma_start(
                                yT_r[:, 2 * mp : 2 * mp + 2, ts(qc, 512)],
                                yst[:])
                        units.append(u)
                    return units

                def attention(qc, pump):
                    q_t = q_ts[qc]
                    nkb = 4 * qc + 4
                    for pr in range(4):
                        po = ps_o.tile([128, 2, 512], F32, tag="ps_o",
                                       name=f"o_{_rep}_{qc}_{pr}")
                        s_tiles = {}
                        p_tiles = {}

                        def s_step(kb, pr=pr, q_t=q_t, s_tiles=s_tiles):
                            r = kb - 4 * qc
                            qlo = _QLO.get(r, 0)
                            s = ps_s.tile([128, 2, 512], F32, tag="ps_s",
                                          name=f"s_{_rep}_{qc}_{pr}_{kb}")
                            for j in (0, 1):
                                pb = 64 * j
                                nc.tensor.matmul(
                                    s[:, j, qlo:512],
                                    k_sb[pb : pb + 64, pr, ts(kb, 128)],
                                    q_t[pb : pb + 64, pr, qlo:512],
                                    start=True, stop=True,
                                    tile_position=(pb, 0),
                                )
                            s_tiles[kb] = (s, qlo)

                        def exp_step(kb, s_tiles=s_tiles, p_tiles=p_tiles,
                                     pr=pr):
                            s, qlo = s_tiles.pop(kb)
                            r = kb - 4 * qc
                            p_t = p_pl.tile([128, 2, 512], BF16, tag="p",
                                            name=f"p_{_rep}_{qc}_{pr}_{kb}")
                            nc.scalar.activation(
                                p_t[:, :, qlo:512], s[:, :, qlo:512],
                                AF.Exp, scale=0.125)
                            if 0 <= r <= 2:
                                c0 = 128 * r
                                nc.gpsimd.tensor_tensor(
                                    p_t[:, :, c0 : c0 + 128],
                                    p_t[:, :, c0 : c0 + 128],
                                    tri[:, None, :].to_broadcast((128, 2, 128)),
                                    OP.mult)
                            elif r == 3:
                                nc.gpsimd.tensor_tensor(
                                    p_t[:, :, 256:512], p_t[:, :, 256:512],
                                    tri2[:, None, :].to_broadcast((128, 2, 256)),
                                    OP.mult)
                            p_tiles[kb] = (p_t, qlo)

                        def av_step(kb, pr=pr, po=po, p_tiles=p_tiles,
                                    nkb=nkb):
                            p_t, qlo = p_tiles.pop(kb)
                            for j in (0, 1):
                                s0 = 2 * pr + j
                                nc.tensor.matmul(
                                    po[:, j, qlo:512],
                                    va[:, kb, s0].rearrange("p a b -> p (a b)"),
                                    p_t[:, j, qlo:512],
                                    start=(kb == 0), stop=(kb == nkb - 1),
                                )

                        s_step(0)
                        if nkb > 1:
                            s_step(1)
                        exp_step(0)
                        for kb in range(nkb):
                            if kb + 2 < nkb:
                                s_step(kb + 2)
                            if kb + 1 < nkb:
                                exp_step(kb + 1)
                            av_step(kb)
                            pump()

                        rec = rc_pl.tile([64, 2, 512], F32, tag="rec",
                                         name=f"rec_{_rep}_{qc}_{pr}")
                        nc.vector.reciprocal(rec[:], po[64:128])
                        for j in (0, 1):
                            nc.vector.tensor_tensor(
                                oT[j * 64 : (j + 1) * 64, pr, ts(qc, 512)],
                                po[0:64, j], rec[:, j], OP.mult)
                        pump()

                # prologue: load x(0)
                x_ts[0] = x_pl.tile([128, CB, 512], F32R, tag="x",
                                    name=f"x_{_rep}_0")
                nc.sync.dma_start(x_ts[0][:], xT_r[:, :, ts(0, 512)])

                for c in range(6):
                    if c + 1 <= 3:
                        x_ts[c + 1] = x_pl.tile([128, CB, 512], F32R, tag="x",
                                                name=f"x_{_rep}_{c + 1}")
                        nc.sync.dma_start(x_ts[c + 1][:],
                                          xT_r[:, :, ts(c + 1, 512)])
                    if c <= 3:
                        q_ts[c] = q_pl.tile([128, 4, 512], F32R, tag="q",
                                            name=f"q_{_rep}_{c}")
                        fill.extend(qkv_units(c))
                    if 2 <= c <= 5:
                        fill.extend(proj_units(c - 2))
                    if 1 <= c <= 4:
                        qc = c - 1
                        steps = 4 * (4 * qc + 4) + 4
                        per = len(fill) / steps
                        state = {"acc": 0.0, "done": 0}

                        def pump(state=state, per=per):
                            state["acc"] += per
                            while state["done"] < int(state["acc"]) and fill:
                                fill.pop(0)()
                                state["done"] += 1

                        attention(qc, pump)
                    while fill:
                        fill.pop(0)()

    nc.compile()
    return nc


def _in_maps(x, W_attn, b_attn, W_proj, b_proj):
    maps = []
    for b in range(B):
        for g in range(2):
            cs = slice(g * 512, (g + 1) * 512)
            maps.append({
                "xT": np.ascontiguousarray(x[b].T),
                "w_qk": np.ascontiguousarray(
                    np.concatenate([W_attn[:, cs], W_attn[:, 1024 + cs.start : 1024 + cs.stop]], axis=1)),
                "w_v": np.ascontiguousarray(W_attn[:, 2048 + cs.start : 2048 + cs.stop]),
                "w_pr": np.ascontiguousarray(W_proj[cs, :]).astype(mybir.dt.np(BF16)),
                "b_qk": np.ascontiguousarray(
                    np.concatenate([b_attn[cs], b_attn[1024 + cs.start : 1024 + cs.stop]])),
                "b_v": np.ascontiguousarray(
                    np.tile(b_attn[2048 + cs.start : 2048 + cs.stop][None, :], (128, 1))),
                "b_pr": np.ascontiguousarray(b_proj),
            })
    return maps


def kernel(x, W_attn, b_attn, W_proj, b_proj):
    x = np.asarray(x, dtype=np.float32)
    W_attn = np.asarray(W_attn, dtype=np.float32)
    b_attn = np.asarray(b_attn, dtype=np.float32)
    W_proj = np.asarray(W_proj, dtype=np.float32)
    b_proj = np.asarray(b_proj, dtype=np.float32)

    if "nc" not in _CACHE:
        _CACHE["nc"] = _build()
    nc = _CACHE["nc"]

    maps = _in_maps(x, W_attn, b_attn, W_proj, b_proj)
    last_exc = None
    for attempt in range(3):
        try:
            res = run_bass_kernel_spmd(nc, maps, core_ids=list(range(N_CORES)))
            break
        except Exception as exc:  # transient device wedges recover on retry
            last_exc = exc
            if attempt == 2:
                raise
            import time as _time
            _time.sleep(5)
    y = np.empty((B, T, C), dtype=np.float32)
    for b in range(B):
        y[b] = (res.results[2 * b]["yT"] + res.results[2 * b + 1]["yT"]).T
    return y
